# revision 7
# baseline (speedup 1.0000x reference)
import numpy as np

import concourse.bacc as bacc
import concourse.mybir as mybir
from concourse.tile import TileContext
from concourse.bass_utils import run_bass_kernel_spmd

B, H, S, D = 1, 12, 2048, 64
NCORES = 8
QS = S // NCORES        # 256 q cols per core
KC = S // 128           # 16 k chunks
W = KC * QS             # 4096 free cols per head tile
FP32 = mybir.dt.float32
FP32R = mybir.dt.float32r
NEG_INF = np.float32(-1e9)

AF = mybir.ActivationFunctionType
ALU = mybir.AluOpType

DVE_MADD_HEADS = {3, 7, 11}

_CACHE = {}


def _build_program():
    nc = bacc.Bacc("TRN2", target_bir_lowering=False)
    scoresT = nc.declare_dram_parameter("scoresT", [H, 128, W], FP32, isOutput=False)
    masknegT = nc.declare_dram_parameter("masknegT", [128, W], FP32, isOutput=False)
    vr = nc.declare_dram_parameter("vr", [128, H, KC, D + 1], FP32R, isOutput=False)
    attnT = nc.declare_dram_parameter("attnT", [H, 128, W], FP32, isOutput=True)
    outT = nc.declare_dram_parameter("outT", [H, D, QS], FP32, isOutput=True)

    with TileContext(nc) as tc:
        with tc.tile_pool(name="constp", bufs=1) as constp, \
             tc.tile_pool(name="vp", bufs=1) as vp, \
             tc.tile_pool(name="mp", bufs=1) as mp, \
             tc.tile_pool(name="mpool", bufs=3) as mpool, \
             tc.tile_pool(name="epool", bufs=2) as epool, \
             tc.tile_pool(name="apool", bufs=2) as apool, \
             tc.tile_pool(name="smallp", bufs=2) as smallp, \
             tc.tile_pool(name="pop", bufs=3, space="PSUM") as pop:

            vt = vp.tile([128, H, KC, D + 1], FP32R)
            nc.sync.dma_start(out=vt, in_=vr[:])
            mt = mp.tile([128, W], FP32)
            nc.sync.dma_start(out=mt, in_=masknegT[:])
            # absorb const-DMA semaphores so per-head ops wait only on
            # their own scores DMA (HW 1-DMA-wait limit)
            sink = constp.tile([128, 1], FP32)
            nc.vector.tensor_copy(out=sink, in_=mt[:, 0:1])
            sink2 = constp.tile([128, 1], FP32R)
            nc.vector.tensor_copy(out=sink2, in_=vt[:, 0, 0, 0:1])

            ms = {}
            es = {}
            pos = {}

            def s_dma(h):
                m = mpool.tile([128, W], FP32, tag="m")
                nc.sync.dma_start(out=m, in_=scoresT[h])
                ms[h] = m

            def s_madd(h):
                eng = nc.vector if h in DVE_MADD_HEADS else nc.gpsimd
                eng.tensor_tensor(out=ms[h], in0=ms[h], in1=mt, op=ALU.add)

            def s_expmm(h):
                e = epool.tile([128, W], FP32R, tag="e")
                nc.scalar.activation(out=e, in_=ms[h], func=AF.Exp)
                po = pop.tile([D + 1, QS], FP32, tag="po")
                for c in range(KC):
                    nc.tensor.matmul(
                        po, vt[:, h, c, :], e[:, c * QS:(c + 1) * QS],
                        start=(c == 0), stop=(c == KC - 1))
                es[h] = e
                pos[h] = po

            def s_tail(h):
                posb = smallp.tile([D + 1, QS], FP32, tag="posb")
                nc.vector.tensor_copy(out=posb, in_=pos[h])
                sbc = smallp.tile([128, QS], FP32, tag="sbc")
                nc.gpsimd.partition_broadcast(sbc, posb[0:1, :])
                rbc = smallp.tile([128, QS], FP32, tag="rbc")
                nc.vector.reciprocal(out=rbc, in_=sbc)
                a = apool.tile([128, W], FP32, tag="a")
                nc.vector.tensor_tensor(
                    out=a.rearrange("p (c q) -> p c q", c=KC),
                    in0=es[h].rearrange("p (c q) -> p c q", c=KC),
                    in1=rbc.unsqueeze(1).broadcast_to([128, KC, QS]),
                    op=ALU.mult)
                nc.gpsimd.dma_start(out=attnT[h], in_=a)
                on = smallp.tile([D + 1, QS], FP32, tag="on")
                nc.vector.tensor_tensor(
                    out=on, in0=posb, in1=rbc[0:D + 1, :], op=ALU.mult)
                nc.gpsimd.dma_start(out=outT[h], in_=on[1:D + 1, :])
                del ms[h], es[h], pos[h]

            for i in range(H + 3):
                if i < H:
                    s_dma(i)
                if 0 <= i - 1 < H:
                    s_madd(i - 1)
                if 0 <= i - 2 < H:
                    s_expmm(i - 2)
                if 0 <= i - 3 < H:
                    s_tail(i - 3)
    return nc


def _get_nc():
    if "nc" not in _CACHE:
        nc = _build_program()
        nc.finalize()
        _CACHE["nc"] = nc
    return _CACHE["nc"]


def run(inputs, trace=False, trace_cores=None):
    v = np.asarray(inputs["v"], dtype=np.float32).reshape(H, S, D)
    ra = np.asarray(inputs["random_attn"], dtype=np.float32).reshape(H, S, S)
    mask = np.asarray(inputs["mask"]).reshape(S, S)

    maskneg_full = mask.astype(np.float32) * NEG_INF  # [S, S]
    vones = np.concatenate(
        [np.ones((H, S, 1), np.float32), v], axis=2)  # [H, S, 1+D]
    vr_host = np.ascontiguousarray(
        vones.reshape(H, KC, 128, D + 1).transpose(2, 0, 1, 3))  # [128,H,KC,D+1]

    in_maps = []
    for c in range(NCORES):
        qlo = c * QS
        sc = np.ascontiguousarray(
            ra[:, qlo:qlo + QS, :].reshape(H, QS, KC, 128)
            .transpose(0, 3, 2, 1)).reshape(H, 128, W)
        mn = np.ascontiguousarray(
            maskneg_full[qlo:qlo + QS].reshape(QS, KC, 128)
            .transpose(2, 1, 0)).reshape(128, W)
        in_maps.append({"scoresT": sc, "masknegT": mn, "vr": vr_host})

    nc = _get_nc()
    kw = {}
    if trace:
        kw["trace"] = True
        if trace_cores is not None:
            kw["trace_cores"] = trace_cores
    res = run_bass_kernel_spmd(nc, in_maps, list(range(NCORES)), **kw)

    attn = np.empty((H, S, S), np.float32)
    out = np.empty((H, S, D), np.float32)
    for c in range(NCORES):
        qlo = c * QS
        r = res.results[c]
        at = np.asarray(r["attnT"]).reshape(H, 128, KC, QS)
        attn[:, qlo:qlo + QS, :] = (
            at.transpose(0, 3, 2, 1).reshape(H, QS, S))
        ot = np.asarray(r["outT"]).reshape(H, D, QS)
        out[:, qlo:qlo + QS, :] = ot.transpose(0, 2, 1)
    return (out.reshape(B, H, S, D), attn.reshape(B, H, S, S)), res.exec_time_ns


def kernel(**inputs):
    return run(inputs)[0]


# revision 8
# speedup vs baseline: 1.0347x; 1.0347x over previous
import numpy as np

import concourse.bacc as bacc
import concourse.mybir as mybir
from concourse.tile import TileContext
from concourse.bass_utils import run_bass_kernel_spmd

B, H, S, D = 1, 12, 2048, 64
NCORES = 8
QS = S // NCORES        # 256 q cols per core
KC = S // 128           # 16 k chunks
W = KC * QS             # 4096 free cols per head tile
FP32 = mybir.dt.float32
FP32R = mybir.dt.float32r
FP16 = mybir.dt.float16
NEG = np.float32(-60000.0)   # exactly representable in fp16; exp -> 0

AF = mybir.ActivationFunctionType
ALU = mybir.AluOpType

CD_MADD = 2944          # DVE handles [0:CD_MADD), Pool the rest
CD_NORM = 2816

_CACHE = {}


def _build_program():
    nc = bacc.Bacc("TRN2", target_bir_lowering=False)
    scoresT = nc.declare_dram_parameter("scoresT", [H, 128, W], FP16, isOutput=False)
    masknegT = nc.declare_dram_parameter("masknegT", [128, W], FP16, isOutput=False)
    vr = nc.declare_dram_parameter("vr", [128, H, KC, D + 1], FP32R, isOutput=False)
    attnT = nc.declare_dram_parameter("attnT", [H, 128, W], FP16, isOutput=True)
    outT = nc.declare_dram_parameter("outT", [H, D, QS], FP32, isOutput=True)

    with TileContext(nc) as tc:
        with tc.tile_pool(name="constp", bufs=1) as constp, \
             tc.tile_pool(name="vp", bufs=1) as vp, \
             tc.tile_pool(name="mp", bufs=1) as mp, \
             tc.tile_pool(name="mpool", bufs=3) as mpool, \
             tc.tile_pool(name="spool", bufs=3) as spool, \
             tc.tile_pool(name="epool", bufs=3) as epool, \
             tc.tile_pool(name="apool", bufs=3) as apool, \
             tc.tile_pool(name="smallp", bufs=2) as smallp, \
             tc.tile_pool(name="pop", bufs=3, space="PSUM") as pop:

            ms = {}
            ss = {}
            es = {}
            pos = {}

            def s_dma(h):
                m = mpool.tile([128, W], FP16, tag="m")
                nc.sync.dma_start(out=m, in_=scoresT[h])
                ms[h] = m

            # order: mask, scores0, then the big 6.4MB v DMA so head-0
            # compute is not queued behind it on the DMA rings
            mt = mp.tile([128, W], FP16)
            nc.sync.dma_start(out=mt, in_=masknegT[:])
            s_dma(0)
            vt = vp.tile([128, H, KC, D + 1], FP32R)
            nc.sync.dma_start(out=vt, in_=vr[:])
            # absorb mask-DMA semaphore so madd waits only on its own
            # scores DMA (HW 1-DMA-wait limit); vt's sem is consumed by
            # the first matmul, which has no other DMA dep
            sink = constp.tile([128, 1], FP32)
            nc.vector.tensor_copy(out=sink, in_=mt[:, 0:1])
            sink3 = constp.tile([128, 1], FP32)
            nc.gpsimd.tensor_copy(out=sink3, in_=mt[:, 1:2])

            def s_madd(h):
                s = spool.tile([128, W], FP16, tag="s")
                nc.vector.tensor_tensor(
                    out=s[:, 0:CD_MADD], in0=ms[h][:, 0:CD_MADD],
                    in1=mt[:, 0:CD_MADD], op=ALU.add)
                nc.gpsimd.tensor_tensor(
                    out=s[:, CD_MADD:W], in0=ms[h][:, CD_MADD:W],
                    in1=mt[:, CD_MADD:W], op=ALU.add)
                ss[h] = s

            def s_expmm(h):
                e = epool.tile([128, W], FP32R, tag="e")
                nc.scalar.activation(out=e, in_=ss[h], func=AF.Exp)
                po = pop.tile([D + 1, QS], FP32, tag="po")
                for c in range(KC):
                    nc.tensor.matmul(
                        po, vt[:, h, c, :], e[:, c * QS:(c + 1) * QS],
                        start=(c == 0), stop=(c == KC - 1))
                es[h] = e
                pos[h] = po

            def s_tail(h):
                posb = smallp.tile([D + 1, QS], FP32, tag="posb")
                nc.vector.tensor_copy(out=posb, in_=pos[h])
                sbc = smallp.tile([128, QS], FP32, tag="sbc")
                nc.gpsimd.partition_broadcast(sbc, posb[0:1, :])
                rbc = smallp.tile([128, QS], FP32, tag="rbc")
                nc.vector.reciprocal_approx_fast(out=rbc, in_=sbc)
                a = apool.tile([128, W], FP16, tag="a")
                e3 = es[h].rearrange("p (c q) -> p c q", c=KC)
                a3 = a.rearrange("p (c q) -> p c q", c=KC)
                b3 = rbc.unsqueeze(1).broadcast_to([128, KC, QS])
                cn = CD_NORM // QS
                nc.vector.tensor_tensor(
                    out=a3[:, 0:cn, :], in0=e3[:, 0:cn, :],
                    in1=b3[:, 0:cn, :], op=ALU.mult)
                nc.gpsimd.tensor_tensor(
                    out=a3[:, cn:KC, :], in0=e3[:, cn:KC, :],
                    in1=b3[:, cn:KC, :], op=ALU.mult)
                on = smallp.tile([D + 1, QS], FP32, tag="on")
                nc.vector.tensor_tensor(
                    out=on, in0=posb, in1=rbc[0:D + 1, :], op=ALU.mult)
                nc.gpsimd.dma_start(out=attnT[h], in_=a)
                nc.sync.dma_start(out=outT[h], in_=on[1:D + 1, :])
                del ms[h], ss[h], es[h], pos[h]

            for i in range(1, H + 3):
                if i < H:
                    s_dma(i)
                if 0 <= i - 1 < H:
                    s_madd(i - 1)
                if 0 <= i - 2 < H:
                    s_expmm(i - 2)
                if 0 <= i - 3 < H:
                    s_tail(i - 3)
    return nc


def _get_nc():
    if "nc" not in _CACHE:
        nc = _build_program()
        nc.finalize()
        _CACHE["nc"] = nc
    return _CACHE["nc"]


def run(inputs, trace=False, trace_cores=None):
    v = np.asarray(inputs["v"], dtype=np.float32).reshape(H, S, D)
    ra = np.asarray(inputs["random_attn"], dtype=np.float32).reshape(H, S, S)
    mask = np.asarray(inputs["mask"]).reshape(S, S)

    maskneg_full = (mask.astype(np.float32) * NEG).astype(np.float16)  # [S, S]
    ra16 = ra.astype(np.float16)
    vones = np.concatenate(
        [np.ones((H, S, 1), np.float32), v], axis=2)  # [H, S, 1+D]
    vr_host = np.ascontiguousarray(
        vones.reshape(H, KC, 128, D + 1).transpose(2, 0, 1, 3))  # [128,H,KC,D+1]

    in_maps = []
    for c in range(NCORES):
        qlo = c * QS
        sc = np.ascontiguousarray(
            ra16[:, qlo:qlo + QS, :].reshape(H, QS, KC, 128)
            .transpose(0, 3, 2, 1)).reshape(H, 128, W)
        mn = np.ascontiguousarray(
            maskneg_full[qlo:qlo + QS].reshape(QS, KC, 128)
            .transpose(2, 1, 0)).reshape(128, W)
        in_maps.append({"scoresT": sc, "masknegT": mn, "vr": vr_host})

    nc = _get_nc()
    kw = {}
    if trace:
        kw["trace"] = True
        if trace_cores is not None:
            kw["trace_cores"] = trace_cores
    res = run_bass_kernel_spmd(nc, in_maps, list(range(NCORES)), **kw)

    attn = np.empty((H, S, S), np.float32)
    out = np.empty((H, S, D), np.float32)
    for c in range(NCORES):
        qlo = c * QS
        r = res.results[c]
        at = np.asarray(r["attnT"]).reshape(H, 128, KC, QS)
        attn[:, qlo:qlo + QS, :] = (
            at.transpose(0, 3, 2, 1).reshape(H, QS, S))
        ot = np.asarray(r["outT"]).reshape(H, D, QS)
        out[:, qlo:qlo + QS, :] = ot.transpose(0, 2, 1)
    return (out.reshape(B, H, S, D), attn.reshape(B, H, S, S)), res.exec_time_ns


def kernel(**inputs):
    return run(inputs)[0]
